# revision 7
# baseline (speedup 1.0000x reference)
"""Multihead self-attention (B=2, S=2048, E=1024, H=16) on 8 trn2 NeuronCores.

Sharding: core c handles batch b = c // 4 and head-group hg = c % 4
(4 heads = 256 projection dims each). Tensor-parallel on heads, data-parallel
on batch. Each core:
  - projects q, k (head-dim-major [f, n]) and v (n-major) for its 4 heads,
  - computes logits^T [kj, qi] per head, exp (fp16), and the AV matmul with a
    fused ones-column that yields softmax row-sums for free,
  - normalizes AV, applies out_proj -> partial output [2048, 1024],
  - ships unnormalized exp (fp16) to the host, which normalizes + head-means
    the attention-weights output.
Host sums partial outputs over head-groups and adds biases.
"""

import os

import numpy as np

P = 128
B = 2
S = 2048
E = 1024
H = 16
D = 64
NH = 4            # heads per core
FS = NH * D       # 256: projection dims per core
ET = E // P       # 8 e-tiles
NT = S // P       # 16 n-tiles / kj-tiles
YH = 1024         # qi half processed per attention pass
SCALING = 0.125   # 1/sqrt(64)

_cached = {}


def _build():
    import concourse.mybir as mybir
    import concourse.tile as tile
    from concourse import bacc

    f32 = mybir.dt.float32
    f32r = mybir.dt.float32r
    f16 = mybir.dt.float16
    Exp = mybir.ActivationFunctionType.Exp

    nc = bacc.Bacc("TRN2", target_bir_lowering=False, debug=False)

    qt_d = nc.dram_tensor("qt", [ET, P, S], f32r, kind="ExternalInput")
    wq_d = nc.dram_tensor("wqt", [ET, P, FS], f32r, kind="ExternalInput")
    wk_d = nc.dram_tensor("wkt", [ET, P, FS], f32r, kind="ExternalInput")
    wv_d = nc.dram_tensor("wvt", [ET, P, FS], f32r, kind="ExternalInput")
    wo_d = nc.dram_tensor("wot", [2, P, E], f32r, kind="ExternalInput")
    bq_d = nc.dram_tensor("bq", [P, 2], f32, kind="ExternalInput")
    bk_d = nc.dram_tensor("bk", [P, 2], f32, kind="ExternalInput")
    bv_d = nc.dram_tensor("bv", [P, 2], f32, kind="ExternalInput")
    exp_d = nc.dram_tensor("exp_out", [NH, NT, P, S], f16, kind="ExternalOutput")
    out_d = nc.dram_tensor("out_part", [NT, P, E], f32, kind="ExternalOutput")

    with tile.TileContext(nc) as tc:
        with tc.tile_pool(name="persist", bufs=1) as pp:
            q_sb = [pp.tile([P, S], f32r, tag=f"q{t}", name=f"q{t}") for t in range(2)]
            k_sb = [pp.tile([P, S], f32r, tag=f"k{t}", name=f"k{t}") for t in range(2)]
            vp = [pp.tile([P, NH * 65], f16, tag=f"vp{n}", name=f"vp{n}") for n in range(NT)]
            ct = [pp.tile([P, S], f32r, tag=f"ct{t}", name=f"ct{t}") for t in range(2)]
            wo_sb = [pp.tile([P, E], f32r, tag=f"wo{t}", name=f"wo{t}") for t in range(2)]
            bq_sb = pp.tile([P, 2], f32, tag="bq", name="bq_sb")
            bk_sb = pp.tile([P, 2], f32, tag="bk", name="bk_sb")
            bv_sb = pp.tile([P, 2], f32, tag="bv", name="bv_sb")
            for t in range(2):
                nc.sync.dma_start(wo_sb[t][:], wo_d[t])
            nc.sync.dma_start(bq_sb[:], bq_d[:])
            nc.sync.dma_start(bk_sb[:], bk_d[:])
            nc.sync.dma_start(bv_sb[:], bv_d[:])

            # ---- phase 1: load + projections ----
            with tc.tile_pool(name="load", bufs=1) as lp:
                qt = [lp.tile([P, S], f32r, tag=f"qt{e}", name=f"qt{e}") for e in range(ET)]
                wq = [lp.tile([P, FS], f32r, tag=f"wq{e}", name=f"wq{e}") for e in range(ET)]
                wk = [lp.tile([P, FS], f32r, tag=f"wk{e}", name=f"wk{e}") for e in range(ET)]
                wv = [lp.tile([P, FS], f32r, tag=f"wv{e}", name=f"wv{e}") for e in range(ET)]
                for e in range(ET):
                    nc.sync.dma_start(qt[e][:], qt_d[e])
                    nc.sync.dma_start(wq[e][:], wq_d[e])
                    nc.sync.dma_start(wk[e][:], wk_d[e])
                    nc.sync.dma_start(wv[e][:], wv_d[e])

                with tc.tile_pool(name="pps", bufs=3, space="PSUM") as pps:
                    # q, k: [f, n] head-dim-major
                    for wmat, dst, bias in ((wq, q_sb, bq_sb), (wk, k_sb, bk_sb)):
                        for t in range(2):
                            for c in range(S // 512):
                                ps = pps.tile([P, 512], f32, tag="projps", name="projps")
                                for e in range(ET):
                                    nc.tensor.matmul(
                                        ps[:],
                                        lhsT=wmat[e][:, t * P:(t + 1) * P],
                                        rhs=qt[e][:, c * 512:(c + 1) * 512],
                                        start=(e == 0),
                                        stop=(e == ET - 1),
                                    )
                                nc.vector.tensor_scalar_add(
                                    dst[t][:, c * 512:(c + 1) * 512], ps[:],
                                    bias[:, t:t + 1],
                                )
                    # v: n-major, bias folded into the AV epilogue
                    for n in range(NT):
                        ps = pps.tile([P, FS], f32, tag="vps", name="vps")
                        for e in range(ET):
                            nc.tensor.matmul(
                                ps[:],
                                lhsT=qt[e][:, n * P:(n + 1) * P],
                                rhs=wv[e][:],
                                start=(e == 0),
                                stop=(e == ET - 1),
                            )
                        for h in range(NH):
                            nc.vector.tensor_copy(
                                vp[n][:, 65 * h:65 * h + 64], ps[:, D * h:D * h + D])
                            nc.vector.memset(vp[n][:, 65 * h + 64:65 * h + 65], 1.0)

            # ---- phase 2: attention ----
            with (
                tc.tile_pool(name="psL", bufs=3, space="PSUM") as plp,
                tc.tile_pool(name="psAV", bufs=1, space="PSUM") as pavp,
                tc.tile_pool(name="et", bufs=4) as etp,
                tc.tile_pool(name="rp", bufs=2) as rp,
            ):
                for h in range(NH):
                    t, off = h // 2, D * (h % 2)
                    qh = q_sb[t][off:off + D, :]
                    kh = k_sb[t][off:off + D, :]
                    for y in range(S // YH):
                        pav = pavp.tile([65, YH], f32, tag="pav", name="pav")
                        for kj in range(NT):
                            pl = plp.tile([P, YH], f32, tag="pl", name="pl")
                            for qc in range(YH // 512):
                                nc.tensor.matmul(
                                    pl[:, qc * 512:(qc + 1) * 512],
                                    lhsT=kh[:, kj * P:(kj + 1) * P],
                                    rhs=qh[:, y * YH + qc * 512:
                                           y * YH + (qc + 1) * 512],
                                    start=True, stop=True,
                                )
                            et_t = etp.tile([P, YH], f16, tag="et", name="et_t")
                            nc.scalar.activation(et_t[:], pl[:], Exp, scale=SCALING)
                            nc.sync.dma_start(
                                exp_d[h, kj, :, y * YH:(y + 1) * YH], et_t[:])
                            for qc in range(YH // 512):
                                nc.tensor.matmul(
                                    pav[:, qc * 512:(qc + 1) * 512],
                                    lhsT=vp[kj][:, 65 * h:65 * h + 65],
                                    rhs=et_t[:, qc * 512:(qc + 1) * 512],
                                    start=(kj == 0), stop=(kj == NT - 1),
                                )
                        rv = rp.tile([1, YH], f32, tag="rv", name="rv")
                        nc.vector.reciprocal(rv[:], pav[64:65, :])
                        rb = rp.tile([D, YH], f32, tag="rb", name="rb")
                        nc.gpsimd.partition_broadcast(rb[:], rv[:], channels=D)
                        dst = ct[t][off:off + D, y * YH:(y + 1) * YH]
                        nc.vector.tensor_mul(dst, pav[0:D, :], rb[:])
                        nc.vector.tensor_scalar_add(
                            dst, dst, bv_sb[off:off + D, t:t + 1])

            # ---- phase 3: out projection ----
            with (
                tc.tile_pool(name="pso", bufs=4, space="PSUM") as pop,
                tc.tile_pool(name="osb", bufs=4) as osp,
            ):
                for n in range(NT):
                    for ec in range(2):
                        ps = pop.tile([P, 512], f32, tag="pso", name="pso")
                        for t in range(2):
                            nc.tensor.matmul(
                                ps[:],
                                lhsT=ct[t][:, n * P:(n + 1) * P],
                                rhs=wo_sb[t][:, ec * 512:(ec + 1) * 512],
                                start=(t == 0), stop=(t == 1),
                            )
                        ob = osp.tile([P, 512], f32, tag="osb", name="ob")
                        nc.vector.tensor_copy(ob[:], ps[:])
                        nc.sync.dma_start(out_d[n, :, ec * 512:(ec + 1) * 512], ob[:])
    nc.compile()
    return nc


def make_in_maps(query, Wq, bq, Wk, bk, Wv, bv, Wo, bo=None):
    query = np.asarray(query, np.float32)
    Wq, Wk, Wv, Wo = (np.asarray(a, np.float32) for a in (Wq, Wk, Wv, Wo))
    bq, bk, bv = (np.asarray(a, np.float32) for a in (bq, bk, bv))
    qt_b = [
        np.ascontiguousarray(query[b].T).reshape(ET, P, S) for b in range(B)
    ]
    in_maps = []
    for c in range(8):
        b, hg = divmod(c, 4)
        fs = slice(FS * hg, FS * (hg + 1))
        in_maps.append({
            "qt": qt_b[b],
            "wqt": np.ascontiguousarray(Wq[fs].T).reshape(ET, P, FS),
            "wkt": np.ascontiguousarray(Wk[fs].T).reshape(ET, P, FS),
            "wvt": np.ascontiguousarray(Wv[fs].T).reshape(ET, P, FS),
            "wot": np.ascontiguousarray(Wo[:, fs].T).reshape(2, P, E),
            "bq": np.ascontiguousarray(bq[fs].reshape(2, P).T),
            "bk": np.ascontiguousarray(bk[fs].reshape(2, P).T),
            "bv": np.ascontiguousarray(bv[fs].reshape(2, P).T),
        })
    return in_maps


def kernel(query, Wq, bq, Wk, bk, Wv, bv, Wo, bo):
    from concourse import bass_utils

    bo = np.asarray(bo, np.float32)
    if "nc" not in _cached:
        _cached["nc"] = _build()
    nc = _cached["nc"]
    in_maps = make_in_maps(query, Wq, bq, Wk, bk, Wv, bv, Wo)

    trace = os.environ.get("BASS_KERNEL_TRACE", "") == "1"
    res = bass_utils.run_bass_kernel_spmd(
        nc, in_maps, core_ids=list(range(8)), trace=trace)
    if trace and res.exec_time_ns is not None:
        print(f"HW exec time: {res.exec_time_ns} ns")

    out = np.broadcast_to(bo, (B, S, E)).astype(np.float32).copy()
    attn = np.zeros((B, S, S), np.float32)
    for c in range(8):
        b = c // 4
        r = res.results[c]
        out[b] += r["out_part"].reshape(S, E)
        e = r["exp_out"].astype(np.float32).reshape(NH, S, S)  # [h, kj, qi]
        rs = e.sum(axis=1, keepdims=True)                      # [h, 1, qi]
        attn[b] += np.matmul(np.ones((1, NH), np.float32),
                             (e / rs).reshape(NH, S * S))[0].reshape(S, S).T
    attn *= 1.0 / H
    return out, attn


# revision 29
# speedup vs baseline: 2.3034x; 2.3034x over previous
"""Multihead self-attention (B=2, S=2048, E=1024, H=16) on 8 trn2 NeuronCores.

Sharding: core c handles batch b = c // 4 and head-group hg = c % 4
(4 heads = 256 projection dims each). Tensor-parallel on heads, data-parallel
on batch. Each core:
  - projects q, k (head-dim-major [f, n]) and v (n-major) for its 4 heads,
  - computes logits^T [kj, qi] per head, exp (fp16), and the AV matmul with a
    fused ones-column that yields softmax row-sums for free,
  - normalizes AV, applies out_proj -> partial output [2048, 1024],
  - ships unnormalized exp (fp16) to the host, which normalizes + head-means
    the attention-weights output.
Host sums partial outputs over head-groups and adds biases.
"""

import os

import numpy as np

P = 128
B = 2
S = 2048
E = 1024
H = 16
D = 64
NH = 4            # heads per core
FS = NH * D       # 256: projection dims per core
ET = E // P       # 8 e-tiles
NT = S // P       # 16 n-tiles / kj-tiles
YH = 1024         # qi half processed per attention pass
SCALING = 0.125   # 1/sqrt(64)

_cached = {}


def _build(reps=1):
    import concourse.mybir as mybir
    import concourse.tile as tile
    from concourse import bacc

    f32 = mybir.dt.float32
    f32r = mybir.dt.float32r
    f16 = mybir.dt.float16
    Exp = mybir.ActivationFunctionType.Exp

    nc = bacc.Bacc("TRN2", target_bir_lowering=False, debug=False)

    qt_d = nc.dram_tensor("qt", [ET, P, S], f16, kind="ExternalInput")
    wq_d = nc.dram_tensor("wqt", [ET, P, FS], f16, kind="ExternalInput")
    wk_d = nc.dram_tensor("wkt", [ET, P, FS], f16, kind="ExternalInput")
    wv_d = nc.dram_tensor("wvt", [ET, P, FS], f16, kind="ExternalInput")
    wo_d = nc.dram_tensor("wot", [2, P, E], f32r, kind="ExternalInput")
    bq_d = nc.dram_tensor("bq", [P, 2], f32, kind="ExternalInput")
    bk_d = nc.dram_tensor("bk", [P, 2], f32, kind="ExternalInput")
    bv_d = nc.dram_tensor("bv", [P, 2], f32, kind="ExternalInput")
    exp_d = nc.dram_tensor("exp_out", [NH, 2, NT, P, YH], f16, kind="ExternalOutput")
    out_d = nc.dram_tensor("out_part", [NT, 2, P, 512], f32, kind="ExternalOutput")

    with tile.TileContext(nc) as tc:
        with (
            tc.tile_pool(name="persist", bufs=1) as pp,
            tc.tile_pool(name="psum", bufs=1, space="PSUM") as psp,
            tc.tile_pool(name="et", bufs=6) as etp,
            tc.tile_pool(name="rp", bufs=2) as rp,
            tc.tile_pool(name="osb", bufs=4) as osp,
        ):
            q_sb = [pp.tile([P, S], f16, tag=f"q{t}", name=f"q{t}")
                    for t in range(2)]
            k_sb = [pp.tile([P, S], f16, tag=f"k{t}", name=f"k{t}")
                    for t in range(2)]
            vp = [pp.tile([P, NH * 65], f16, tag=f"vp{n}", name=f"vp{n}")
                  for n in range(NT)]
            ct = [pp.tile([P, S], f32r, tag=f"ct{t}", name=f"ct{t}")
                  for t in range(2)]
            wo_sb = [pp.tile([P, E], f32r, tag=f"wo{t}", name=f"wo{t}")
                     for t in range(2)]
            bq_sb = pp.tile([P, 2], f32, tag="bq", name="bq_sb")
            bk_sb = pp.tile([P, 2], f32, tag="bk", name="bk_sb")
            bv_sb = pp.tile([P, 2], f32, tag="bv", name="bv_sb")
            qt = [pp.tile([P, S], f16, tag=f"qt{e}", name=f"qt{e}")
                  for e in range(ET)]
            wq = [pp.tile([P, FS], f16, tag=f"wq{e}", name=f"wq{e}")
                  for e in range(ET)]
            wk = [pp.tile([P, FS], f16, tag=f"wk{e}", name=f"wk{e}")
                  for e in range(ET)]
            wv = [pp.tile([P, FS], f16, tag=f"wv{e}", name=f"wv{e}")
                  for e in range(ET)]

            for _rep in range(reps):
                for t in range(2):
                    nc.sync.dma_start(wo_sb[t][:], wo_d[t])
                nc.sync.dma_start(bq_sb[:], bq_d[:])
                nc.sync.dma_start(bk_sb[:], bk_d[:])
                nc.sync.dma_start(bv_sb[:], bv_d[:])
                for e in range(ET):
                    nc.sync.dma_start(qt[e][:], qt_d[e])
                    nc.sync.dma_start(wq[e][:], wq_d[e])
                    nc.sync.dma_start(wk[e][:], wk_d[e])
                    nc.sync.dma_start(wv[e][:], wv_d[e])

                # ---- phase 1: projections (emitted interleaved with
                # attention: heads 0/1 only need the t=0 slices) ----
                def proj_chain(wmat, dst, bias, t, c):
                    ps = psp.tile([P, 1024], f32, tag="pl", bufs=3,
                                  name="projps")
                    for e in range(ET):
                        nc.tensor.matmul(
                            ps[:, 0:512],
                            lhsT=wmat[e][:, t * P:(t + 1) * P],
                            rhs=qt[e][:, c * 512:(c + 1) * 512],
                            start=(e == 0),
                            stop=(e == ET - 1),
                        )
                    nc.vector.tensor_scalar_add(
                        dst[t][:, c * 512:(c + 1) * 512],
                        ps[:, 0:512], bias[:, t:t + 1],
                    )

                def proj_qk(t):
                    for wmat, dst, bias in ((wq, q_sb, bq_sb),
                                            (wk, k_sb, bk_sb)):
                        for c in range(S // 512):
                            proj_chain(wmat, dst, bias, t, c)

                def v_chain(n):
                    ps = psp.tile([P, 1024], f32, tag="pl", bufs=3,
                                  name="vps")
                    for e in range(ET):
                        nc.tensor.matmul(
                            ps[:, 0:FS],
                            lhsT=qt[e][:, n * P:(n + 1) * P],
                            rhs=wv[e][:],
                            start=(e == 0),
                            stop=(e == ET - 1),
                        )
                    for h in range(NH):
                        nc.vector.tensor_copy(
                            vp[n][:, 65 * h:65 * h + 64],
                            ps[:, D * h:D * h + D])
                        nc.vector.memset(
                            vp[n][:, 65 * h + 64:65 * h + 65], 1.0)

                proj_qk(0)
                pending_v = list(range(NT))
                for n in range(4):
                    v_chain(pending_v.pop(0))

                # out-projection chain for one (n, ec) output block; half
                # y's chains are interleaved into half y+1's attention
                # emission so they fill PE/psum gaps instead of bursting.
                def out_proj_chain(n, ec):
                    ps = psp.tile([P, 1024], f32, tag="pl", bufs=3,
                                  name="pso")
                    for t in range(2):
                        nc.tensor.matmul(
                            ps[:, 0:512],
                            lhsT=ct[t][:, n * P:(n + 1) * P],
                            rhs=wo_sb[t][:, ec * 512:(ec + 1) * 512],
                            start=(t == 0), stop=(t == 1),
                        )
                    ob = osp.tile([P, 512], f32, tag="osb", name="ob")
                    nc.vector.tensor_copy(ob[:], ps[:, 0:512])
                    nc.sync.dma_start(out_d[n, ec], ob[:])

                pending = []

                # ---- phase 2: attention ----
                pending_p = [(wm, ds, bi, 1, c)
                             for wm, ds, bi in ((wq, q_sb, bq_sb),
                                                (wk, k_sb, bk_sb))
                             for c in range(S // 512)]
                for y in range(S // YH):
                    for h in range(NH):
                        if y == 0 and h == 2 and pending_p:
                            for args in pending_p:
                                proj_chain(*args)
                            pending_p = []
                        t, off = h // 2, D * (h % 2)
                        qh = q_sb[t][off:off + D, :]
                        kh = k_sb[t][off:off + D, :]
                        pav = psp.tile([65, YH], f32, tag="pav", name="pav")
                        for kj in range(NT):
                            if pending_v:
                                # produce vp[kj+2] just ahead of AV's use
                                while pending_v and pending_v[0] <= kj + 2:
                                    v_chain(pending_v.pop(0))
                            elif pending_p and kj % 2 == 0:
                                proj_chain(*pending_p.pop(0))
                            pl = psp.tile([P, YH], f32, tag="pl", bufs=3, name="pl")
                            for qc in range(YH // 512):
                                nc.tensor.matmul(
                                    pl[:, qc * 512:(qc + 1) * 512],
                                    lhsT=kh[:, kj * P:(kj + 1) * P],
                                    rhs=qh[:, y * YH + qc * 512:
                                           y * YH + (qc + 1) * 512],
                                    start=True, stop=True,
                                )
                            et_t = etp.tile([P, YH], f16, tag="et",
                                            name="et_t")
                            nc.scalar.activation(et_t[:], pl[:], Exp,
                                                 scale=SCALING)
                            nc.sync.dma_start(
                                exp_d[h, y, kj], et_t[:])
                            for qc in range(YH // 512):
                                nc.tensor.matmul(
                                    pav[:, qc * 512:(qc + 1) * 512],
                                    lhsT=vp[kj][:, 65 * h:65 * h + 65],
                                    rhs=et_t[:, qc * 512:(qc + 1) * 512],
                                    start=(kj == 0), stop=(kj == NT - 1),
                                )
                            if pending and kj % 2 == 1:
                                out_proj_chain(*pending.pop(0))
                        rv = rp.tile([1, YH], f32, tag="rv", name="rv")
                        nc.vector.reciprocal(rv[:], pav[64:65, :])
                        rb = rp.tile([D, YH], f32, tag="rb", name="rb")
                        nc.gpsimd.partition_broadcast(rb[:], rv[:],
                                                      channels=D)
                        dst = ct[t][off:off + D, y * YH:(y + 1) * YH]
                        nc.vector.tensor_mul(dst, pav[0:D, :], rb[:])
                        nc.vector.tensor_scalar_add(
                            dst, dst, bv_sb[off:off + D, t:t + 1])
                    # queue this half's out-proj; emitted during next half
                    pending.extend(
                        (n, ec)
                        for n in range(y * (NT // 2), (y + 1) * (NT // 2))
                        for ec in range(2))
                for n, ec in pending:
                    out_proj_chain(n, ec)
    nc.compile()
    return nc


def make_in_maps(query, Wq, bq, Wk, bk, Wv, bv, Wo, bo=None):
    query = np.asarray(query, np.float32)
    Wq, Wk, Wv, Wo = (np.asarray(a, np.float32) for a in (Wq, Wk, Wv, Wo))
    bq, bk, bv = (np.asarray(a, np.float32) for a in (bq, bk, bv))
    qt_b = [
        np.ascontiguousarray(query[b].T).reshape(ET, P, S) for b in range(B)
    ]
    in_maps = []
    for c in range(8):
        b, hg = divmod(c, 4)
        fs = slice(FS * hg, FS * (hg + 1))
        in_maps.append({
            "qt": qt_b[b].astype(np.float16),
            "wqt": np.ascontiguousarray(Wq[fs].T).reshape(ET, P, FS)
                     .astype(np.float16),
            "wkt": np.ascontiguousarray(Wk[fs].T).reshape(ET, P, FS)
                     .astype(np.float16),
            "wvt": np.ascontiguousarray(Wv[fs].T).reshape(ET, P, FS)
                     .astype(np.float16),
            "wot": np.ascontiguousarray(Wo[:, fs].T).reshape(2, P, E),
            "bq": np.ascontiguousarray(bq[fs].reshape(2, P).T),
            "bk": np.ascontiguousarray(bk[fs].reshape(2, P).T),
            "bv": np.ascontiguousarray(bv[fs].reshape(2, P).T),
        })
    return in_maps


def kernel(query, Wq, bq, Wk, bk, Wv, bv, Wo, bo):
    from concourse import bass_utils

    bo = np.asarray(bo, np.float32)
    if "nc" not in _cached:
        _cached["nc"] = _build()
    nc = _cached["nc"]
    in_maps = make_in_maps(query, Wq, bq, Wk, bk, Wv, bv, Wo)

    trace = os.environ.get("BASS_KERNEL_TRACE", "") == "1"
    res = bass_utils.run_bass_kernel_spmd(
        nc, in_maps, core_ids=list(range(8)), trace=trace)
    if trace and res.exec_time_ns is not None:
        print(f"HW exec time: {res.exec_time_ns} ns")

    out = np.broadcast_to(bo, (B, S, E)).astype(np.float32).copy()
    attn = np.zeros((B, S, S), np.float32)
    for c in range(8):
        b = c // 4
        r = res.results[c]
        out[b] += r["out_part"].transpose(0, 2, 1, 3).reshape(S, E)
        # [h, y, kj, p, q] -> [h, kj*p, y*q]
        e = r["exp_out"].astype(np.float32).transpose(0, 2, 3, 1, 4) \
            .reshape(NH, S, S)
        rs = e.sum(axis=1, keepdims=True)                      # [h, 1, qi]
        attn[b] += (e / rs).sum(axis=0).T
    attn *= 1.0 / H
    return out, attn
